# revision 15
# baseline (speedup 1.0000x reference)
"""Trainium2 Bass kernel for a 3-type heterogeneous GraphSAGE GNN.

Full-input contract: kernel(**inputs) takes the unsharded numpy inputs and
returns the full [300000, 2] output. Internally:
  - Nodes are relabeled so each of the 8 cores owns a contiguous range of
    37632 padded nodes (12544 per type); edges are sharded by dst owner.
  - Per core, edges are sorted by (src block of 32768, dst) and padded into
    a schedule of 128-edge chunks that is *uniform across cores* (the NEFF
    is SPMD — one program, per-core data).
  - Aggregation: dma_gather pulls x[src] rows (256B) from a replicated
    x_full in DRAM; a 0/1 one-hot [128 edges x 128 dsts] built on DVE via
    is_equal(iota, dstrel) turns segment-sum into PE matmuls accumulated
    in PSUM.
  - Schedule: pass 1 covers block 0 for all banks (runs while the
    replicated encoder still produces later blocks); pass 2 is bank-outer
    (for bank: for blocks 1..9) so each bank of 8 dst-groups finishes
    early and its mean/post-linear/classifier work pipelines on PE/DVE/Act
    underneath the GpSimd gather stream.
  - Layer-0 x_full is produced by a replicated encoder (every core encodes
    all nodes; no collective); layer-1 x_full needs one AllGather.
"""

import numpy as np

import concourse.bass as bass
import concourse.bacc as bacc
import concourse.mybir as mybir
import concourse.tile as tile
from concourse.masks import make_identity
from concourse.bass_utils import run_bass_kernel_spmd

F32 = mybir.dt.float32
I16 = mybir.dt.int16

FULL_CFG = dict(type_size=100000, E=4800000, cores=8, blk=32768,
                strip_chunks=32, h=64)

KENC = 49      # 48 padded features + ones column
BANKW = 4      # dst-groups per bank (one PSUM bank per in-flight chain)
PBANKF = 512   # f32 per PSUM bank per partition


def derive(cfg):
    cores = cfg["cores"]
    seg = cfg["type_size"] // cores          # real nodes per (core, type)
    assert seg * cores == cfg["type_size"]
    segp = -(-seg // 128) * 128              # padded to tile multiple
    npc = 3 * segp                           # nodes per core (padded)
    nptot = cores * npc
    tiles = npc // 128
    groups = tiles                           # 128-dst groups per core
    nblk = -(-nptot // cfg["blk"])
    nbank = -(-groups // BANKW)
    d = dict(cfg)
    d.update(seg=seg, segp=segp, npc=npc, nptot=nptot, tiles=tiles,
             groups=groups, nblk=nblk, nbank=nbank)
    return d


def node_perm(d):
    """perm_of_orig[j] = padded-global id of original node j."""
    ts, seg, segp, npc = d["type_size"], d["seg"], d["segp"], d["npc"]
    j = np.arange(ts)
    core = j // seg
    local = j % seg
    parts = [core * npc + t * segp + local for t in range(3)]
    return np.concatenate(parts)


class Sched:
    pass


def plan(d, edge_index):
    """Build the uniform schedule + per-core edge data arrays."""
    cores, npc, nptot, blk = d["cores"], d["npc"], d["nptot"], d["blk"]
    groups, nblk, sc = d["groups"], d["nblk"], d["strip_chunks"]
    nbank = d["nbank"]

    perm = node_perm(d)
    src_p = perm[np.asarray(edge_index[0], dtype=np.int64)]
    dst_p = perm[np.asarray(edge_index[1], dtype=np.int64)]

    deg = np.bincount(dst_p, minlength=nptot).astype(np.float64)
    winv_full = (1.0 / np.maximum(deg, 1.0)).astype(np.float32)

    # per-core sorted edge arrays + per-(block, group) counts
    core_of = dst_p // npc
    per_core = []
    counts = np.zeros((cores, nblk, groups), np.int64)
    for c in range(cores):
        m = core_of == c
        es = src_p[m]
        ed = dst_p[m] - c * npc
        b = es // blk
        order = np.lexsort((ed, b))
        es, ed, b = es[order], ed[order], b[order]
        g = ed // 128
        np.add.at(counts[c], (b, g), 1)
        per_core.append((es, ed))

    nch = np.maximum(1, -(-counts.max(axis=0) // 128))  # [nblk, groups]

    # ---- chunk list in program order ----
    # pass 1: (b=0, bank k, g in bank, j)    -- block 0 for every bank
    # pass 2: (bank k, b=1..nblk-1, g, j)    -- bank-outer for the rest
    # A group's psum accumulation: start at (b=0, j=0); b=0 partials are
    # flushed (copied) to agg per bank; pass-2 restarts psum per bank and
    # the final fused flush adds psum into agg then multiplies by winv.
    assert nblk >= 2
    chunks = []  # (b, g, start, stop)
    for k in range(nbank):
        g_lo, g_hi = k * BANKW, min((k + 1) * BANKW, groups)
        for g in range(g_lo, g_hi):
            n = int(nch[0, g])
            for j in range(n):
                chunks.append((0, g, j == 0, j == n - 1))
    for k in range(nbank):
        g_lo, g_hi = k * BANKW, min((k + 1) * BANKW, groups)
        for b in range(1, nblk):
            for g in range(g_lo, g_hi):
                n = int(nch[b, g])
                for j in range(n):
                    chunks.append((b, g, b == 1 and j == 0,
                                   b == nblk - 1 and j == n - 1))
    nchunks = len(chunks)
    chunk_b = np.array([c[0] for c in chunks])
    chunk_g = np.array([c[1] for c in chunks])

    # ---- strips: runs of consecutive chunks sharing b, cut at sc ----
    strips = []  # (b, c0, n, idx_col_off)
    idx_off = 0
    c0 = 0
    for ci in range(nchunks + 1):
        if ci == nchunks or (ci > c0 and chunk_b[ci] != chunk_b[c0]) \
           or ci - c0 == sc:
            if ci > c0:
                n = ci - c0
                strips.append((int(chunk_b[c0]), c0, n, idx_off))
                idx_off += n * 8
            c0 = ci
    idx_cols = idx_off
    strip_of_chunk = np.zeros(nchunks, np.int64)
    strip_c0 = np.zeros(nchunks, np.int64)
    for si, (b, sc0, n, io) in enumerate(strips):
        strip_of_chunk[sc0:sc0 + n] = si
        strip_c0[sc0:sc0 + n] = sc0

    # ---- op list: strips/mms/bank-end markers in program order ----
    # ops: ("strip", si) | ("mm", si, k_in_strip, g, start, stop)
    #      | ("flush0", bank)  -- after pass-1 bank: copy psum -> agg
    #      | ("flushw", bank)  -- after pass-2 bank: agg=(agg+psum)*winv
    #      | ("post", bank)    -- post-linear for the bank (layer-specific)
    ops = []
    for si, (b, sc0, n, io) in enumerate(strips):
        ops.append(("strip", si))
        for kk in range(n):
            cb, cg, cst, csp = chunks[sc0 + kk]
            ops.append(("mm", si, kk, cg, cst, csp))
            nxt = sc0 + kk + 1
            bank = cg // BANKW
            g_hi = min((bank + 1) * BANKW, groups) - 1
            if cb == 0 and csp:
                # pass-1: after last chunk of this bank's b=0 run
                is_end = (nxt == nchunks or chunk_b[nxt] != 0
                          or chunk_g[nxt] // BANKW != bank)
                if is_end:
                    ops.append(("flush0", bank))
            if cb == nblk - 1 and csp and cg == g_hi:
                ops.append(("flushw", bank))
                ops.append(("post", bank))

    # ---- per-core data arrays ----
    # slots: chunk ci occupies slots [ci*128, (ci+1)*128)
    # cell (b,g) slot ranges: consecutive chunks of the cell
    cell_first_chunk = {}
    for ci2, (b, g, st, sp) in enumerate(chunks):
        if st or (b, g) not in cell_first_chunk:
            pass
        if (b, g) not in cell_first_chunk:
            cell_first_chunk[(b, g)] = ci2
    # within-cell chunk index
    within_chunk = np.zeros(nchunks, np.int64)
    seen = {}
    for ci2, (b, g, st, sp) in enumerate(chunks):
        key = (b, g)
        within_chunk[ci2] = seen.get(key, 0)
        seen[key] = within_chunk[ci2] + 1

    slot = np.arange(nchunks * 128)
    ch_of_slot = slot // 128
    lane = slot % 128
    slot_b = chunk_b[ch_of_slot]
    slot_g = chunk_g[ch_of_slot]
    within = within_chunk[ch_of_slot] * 128 + lane   # position within cell

    strip_local = (ch_of_slot - strip_c0[ch_of_slot]) * 128 + lane
    idx_col = np.array([strips[s][3] for s in strip_of_chunk[ch_of_slot]]) \
        + strip_local // 16
    idx_row = strip_local % 16

    # per-cell edge start offsets in the per-core sorted arrays
    # (sorted order is b-major then dst => cell order (b, g) b-major)
    idx_all = np.zeros((cores, 128, idx_cols), np.int16)
    dstrel_all = np.full((cores, 128, nchunks), -1.0, np.float32)
    for c in range(cores):
        es, ed = per_core[c]
        ccounts = counts[c]                       # [nblk, groups]
        flat = ccounts.ravel()
        cell_start = np.concatenate([[0], np.cumsum(flat)])
        cell_id = slot_b * groups + slot_g
        cnt = flat[cell_id]
        real = within < cnt
        src_idx = cell_start[cell_id] + np.minimum(
            within, np.maximum(cnt - 1, 0))
        esv = np.where(real, es[np.minimum(src_idx, max(len(es) - 1, 0))]
                       if len(es) else 0, 0)
        edv = np.where(real, ed[np.minimum(src_idx, max(len(ed) - 1, 0))]
                       if len(ed) else 0, -1)
        rel = np.where(real, esv - slot_b * blk, 0).astype(np.int64)
        assert rel.min() >= 0 and rel.max() < blk
        drel = np.where(real, edv - slot_g * 128, -1.0).astype(np.float32)
        for r in range(8):
            idx_all[c, idx_row + 16 * r, idx_col] = rel.astype(np.int16)
        dstrel_all[c, lane, ch_of_slot] = drel

    s = Sched()
    s.d = d
    s.perm = perm
    s.strips = strips
    s.ops = ops
    s.nchunks = nchunks
    s.idx_cols = idx_cols
    s.winv_full = winv_full
    s.idx_all = idx_all
    s.dstrel_all = dstrel_all
    return s


def core_inputs(s, x_individual, x_company, x_trust,
                W_ind, b_ind, W_com, b_com, W_tru, b_tru,
                W1l, W1r, b1, W2l, W2r, b2, Wc1, bc1, Wc2, bc2):
    d = s.d
    cores, seg, segp, npc, groups = \
        d["cores"], d["seg"], d["segp"], d["npc"], d["groups"]
    raws = [np.asarray(x_individual, np.float32),
            np.asarray(x_company, np.float32),
            np.asarray(x_trust, np.float32)]
    Ws = [np.asarray(W_ind, np.float32), np.asarray(W_com, np.float32),
          np.asarray(W_tru, np.float32)]
    bs = [np.asarray(b_ind, np.float32), np.asarray(b_com, np.float32),
          np.asarray(b_tru, np.float32)]
    h = d["h"]

    w_enc = []
    for t in range(3):
        w = np.zeros((KENC, h), np.float32)
        w[:Ws[t].shape[0], :] = Ws[t]
        w[48, :] = bs[t]
        w_enc.append(w)

    # full padded raw features in permuted layout [nptot, KENC]
    nptot = d["nptot"]
    xraw_full = np.zeros((nptot, KENC), np.float32)
    for c in range(cores):
        for t in range(3):
            r0 = c * npc + t * segp
            xraw_full[r0:r0 + seg, :raws[t].shape[1]] = \
                raws[t][c * seg:(c + 1) * seg]
            xraw_full[r0:r0 + seg, 48] = 1.0

    shared = {
        "xraw_full": xraw_full,
        "w_enc0": w_enc[0], "w_enc1": w_enc[1], "w_enc2": w_enc[2],
        "w1l": np.asarray(W1l, np.float32), "w1r": np.asarray(W1r, np.float32),
        "w2l": np.asarray(W2l, np.float32), "w2r": np.asarray(W2r, np.float32),
        "wc1": np.asarray(Wc1, np.float32), "wc2": np.asarray(Wc2, np.float32),
        "b1_rep": np.tile(np.asarray(b1, np.float32)[None, :], (128, 1)),
        "b2_rep": np.tile(np.asarray(b2, np.float32)[None, :], (128, 1)),
        "bc1_rep": np.tile(np.asarray(bc1, np.float32)[None, :], (128, 1)),
        "bc2_rep": np.tile(np.asarray(bc2, np.float32)[None, :], (128, 1)),
        "iota_rep": np.tile(np.arange(128, dtype=np.float32)[None, :],
                            (128, 1)),
    }

    in_maps = []
    for c in range(cores):
        xraw = np.zeros((npc, KENC), np.float32)
        for t in range(3):
            r0 = t * segp
            xraw[r0:r0 + seg, :raws[t].shape[1]] = \
                raws[t][c * seg:(c + 1) * seg]
            xraw[r0:r0 + seg, 48] = 1.0
        winv = s.winv_full[c * npc:(c + 1) * npc] \
            .reshape(groups, 128).T.copy()
        m = dict(shared)
        m.update(xraw=xraw, idx=s.idx_all[c], dstrel=s.dstrel_all[c],
                 winv=winv)
        in_maps.append(m)
    return in_maps


def build_program(s, debug_dump=False):
    d = s.d
    cores, npc, nptot, blk = d["cores"], d["npc"], d["nptot"], d["blk"]
    tiles, groups, nblk, h = d["tiles"], d["groups"], d["nblk"], d["h"]
    nbank, sc = d["nbank"], d["strip_chunks"]

    nc = bacc.Bacc("TRN2", target_bir_lowering=False, debug=False,
                   num_devices=cores, dynamic_dma_scratch_size=32768)

    di = {}
    def inp(name, shape, dt=F32):
        di[name] = nc.dram_tensor(name, list(shape), dt, kind="ExternalInput")
        return di[name]

    inp("xraw", [npc, KENC])
    inp("xraw_full", [nptot, KENC])
    inp("idx", [128, s.idx_cols], I16)
    inp("dstrel", [128, s.nchunks])
    inp("winv", [128, groups])
    inp("iota_rep", [128, 128])
    for t in range(3):
        inp(f"w_enc{t}", [KENC, h])
    inp("w1l", [h, h]); inp("w1r", [h, h]); inp("b1_rep", [128, h])
    inp("w2l", [h, h]); inp("w2r", [h, h]); inp("b2_rep", [128, h])
    inp("wc1", [h, 32]); inp("bc1_rep", [128, 32])
    inp("wc2", [32, 2]); inp("bc2_rep", [128, 2])
    out_d = nc.dram_tensor("out", [npc, 2], F32, kind="ExternalOutput")
    dbg_xf0 = dbg_xo1 = None
    if debug_dump:
        dbg_xf0 = nc.dram_tensor("dbg_xf0", [nptot, h], F32,
                                 kind="ExternalOutput")
        dbg_xo1 = nc.dram_tensor("dbg_xo1", [npc, h], F32,
                                 kind="ExternalOutput")
        dbg_xo0 = nc.dram_tensor("dbg_xo0", [npc, h], F32,
                                 kind="ExternalOutput")
        dbg_mean = nc.dram_tensor("dbg_mean", [npc, h], F32,
                                  kind="ExternalOutput")

    AG = "AllGather"
    ADD = mybir.AluOpType.add
    MUL = mybir.AluOpType.mult
    EQ = mybir.AluOpType.is_equal
    BYP = mybir.AluOpType.bypass
    RELU = mybir.ActivationFunctionType.Relu

    blk_rows = [min(blk, nptot - b * blk) for b in range(nblk)]

    with tile.TileContext(nc) as tc:
        with tc.tile_pool(name="persist", bufs=1) as pp, \
             tc.tile_pool(name="dram", bufs=1, space="DRAM") as dramp:
            # constants to SBUF
            def csb(name, shape, dt=F32):
                t_ = pp.tile(list(shape), dt, tag=name)
                nc.sync.dma_start(t_[:], di[name].ap())
                return t_
            iota_sb = csb("iota_rep", [128, 128])
            winv_sb = csb("winv", [128, groups])
            wenc_sb = [csb(f"w_enc{t}", [KENC, h]) for t in range(3)]
            wl_sb = [csb("w1l", [h, h]), csb("w2l", [h, h])]
            wr_sb = [csb("w1r", [h, h]), csb("w2r", [h, h])]
            brep_sb = [csb("b1_rep", [128, h]), csb("b2_rep", [128, h])]
            wc1_sb = csb("wc1", [h, 32])
            bc1_sb = csb("bc1_rep", [128, 32])
            wc2_sb = csb("wc2", [32, 2])
            bc2_sb = csb("bc2_rep", [128, 2])
            ident = pp.tile([128, 128], F32, tag="ident")
            make_identity(nc, ident[:])

            agg = pp.tile([128, groups * h], F32, tag="agg")
            outb = pp.tile([128, tiles * 2], F32, tag="outb")

            x_own0 = dramp.tile([npc, h], F32)
            x_own1 = dramp.tile([npc, h], F32)
            xf0 = [dramp.tile([blk_rows[b], h], F32, name=f"xf0_{b}")
                   for b in range(nblk)]
            xf1 = dramp.tile([nptot, h], F32, name="xf1")

            # ---------------- replicated encoder -> xf0 blocks ----------
            segp = d["segp"]
            with tc.tile_pool(name="encio", bufs=2) as pio, \
                 tc.tile_pool(name="enc", bufs=4) as pe, \
                 tc.tile_pool(name="encps", bufs=4, space="PSUM") as pse:
                QT = 64                       # tiles per quarter-block load
                for b in range(nblk):
                    tb = blk_rows[b] // 128
                    for q0 in range(0, tb, QT):
                        nt = min(QT, tb - q0)
                        xr = pio.tile([128, QT * KENC], F32, tag="xr")
                        r0 = b * blk + q0 * 128
                        nc.sync.dma_start(
                            xr[:, :nt * KENC].rearrange(
                                "p (t f) -> p t f", f=KENC),
                            di["xraw_full"].ap()[r0:r0 + nt * 128, :]
                            .rearrange("(t p) f -> p t f", p=128))
                        eb = pio.tile([128, QT * h], F32, tag="eb")
                        for t in range(nt):
                            gt = (r0 + t * 128) % npc // segp  # node type
                            tp = pse.tile([64, 128], F32, tag="tp")
                            nc.tensor.transpose(
                                out=tp[:KENC, :],
                                in_=xr[:, t * KENC:(t + 1) * KENC],
                                identity=ident[:])
                            xrT = pe.tile([KENC, 128], F32, tag="xrT")
                            nc.scalar.copy(xrT[:], tp[:KENC, :])
                            ym = pse.tile([128, h], F32, tag="ym")
                            nc.tensor.matmul(out=ym[:], lhsT=xrT[:],
                                             rhs=wenc_sb[gt][:],
                                             start=True, stop=True)
                            nc.scalar.copy(eb[:, t * h:(t + 1) * h], ym[:])
                        nc.sync.dma_start(
                            xf0[b][q0 * 128:q0 * 128 + nt * 128, :]
                            .rearrange("(t p) f -> p t f", p=128),
                            eb[:, :nt * h].rearrange("p (t f) -> p t f", f=h))

            # ---------------- own-shard encoder -> x_own0 ----------------
            with tc.tile_pool(name="enc2io", bufs=1) as pio2, \
                 tc.tile_pool(name="enc2", bufs=4) as pe2, \
                 tc.tile_pool(name="enc2ps", bufs=4, space="PSUM") as pse2:
                xraw_sb = pio2.tile([128, tiles * KENC], F32)
                nc.sync.dma_start(
                    xraw_sb[:].rearrange("p (t f) -> p t f", f=KENC),
                    di["xraw"].ap().rearrange("(t p) f -> p t f", p=128))
                seg_tiles = segp // 128
                for t in range(tiles):
                    wseg = wenc_sb[t // seg_tiles]
                    tp = pse2.tile([64, 128], F32, tag="tp")
                    nc.tensor.transpose(
                        out=tp[:KENC, :],
                        in_=xraw_sb[:, t * KENC:(t + 1) * KENC],
                        identity=ident[:])
                    xrT = pe2.tile([KENC, 128], F32, tag="xrT")
                    nc.scalar.copy(xrT[:], tp[:KENC, :])
                    ym = pse2.tile([128, h], F32, tag="ym")
                    nc.tensor.matmul(out=ym[:], lhsT=xrT[:], rhs=wseg[:],
                                     start=True, stop=True)
                    nc.scalar.copy(agg[:, t * h:(t + 1) * h], ym[:])
                nc.sync.dma_start(
                    x_own0[:, :].rearrange("(t p) f -> p t f", p=128),
                    agg[:].rearrange("p (t f) -> p t f", f=h))

            # ---------------- SAGE layers ----------------
            x_own = [x_own0, x_own1]
            for L in range(2):
                def in_ap_of(b):
                    if L == 0:
                        return xf0[b][:, :]
                    return xf1[b * blk:b * blk + blk_rows[b], :]

                with tc.tile_pool(name=f"sage{L}", bufs=2) as pa, \
                     tc.tile_pool(name=f"sageps{L}", bufs=1,
                                  space="PSUM") as psa, \
                     tc.tile_pool(name=f"post{L}", bufs=4) as ppo, \
                     tc.tile_pool(name=f"post{L}ps", bufs=1,
                                  space="PSUM") as psp, \
                     tc.tile_pool(name=f"post{L}io", bufs=2) as pio3:
                    psum_agg = psa.tile([128, BANKW * PBANKF], F32)
                    # pre-zero message buffers (stale SBUF could hold NaNs;
                    # one-hot zero-rows would still propagate 0*NaN)
                    msgs_bufs = [pa.tile([128, sc * h], F32, tag="msgs",
                                         name=f"msgsbuf{L}_{i}")
                                 for i in range(2)]
                    for mb in msgs_bufs:
                        nc.vector.memset(mb[:], 0.0)
                    cur = {}
                    mi = 0
                    for op in s.ops:
                        if op[0] == "strip":
                            si = op[1]
                            b, c0, n, ioff = s.strips[si]
                            idx_sb = pa.tile([128, sc * 8], I16, tag="idx")
                            nc.sync.dma_start(
                                idx_sb[:, :n * 8],
                                di["idx"].ap()[:, ioff:ioff + n * 8])
                            dst_sb = pa.tile([128, sc], F32, tag="dst")
                            nc.sync.dma_start(
                                dst_sb[:, :n],
                                di["dstrel"].ap()[:, c0:c0 + n])
                            msgs = msgs_bufs[mi % 2]
                            mi += 1
                            oh = pa.tile([128, sc * 128], F32, tag="oh")
                            nc.gpsimd.dma_gather(
                                out_ap=msgs[:, :n * h].rearrange(
                                    "p (c f) -> p c f", f=h),
                                in_ap=in_ap_of(b),
                                idxs_ap=idx_sb[:, :n * 8],
                                num_idxs=n * 128, num_idxs_reg=n * 128,
                                elem_size=h, single_packet=False)
                            nc.vector.tensor_tensor(
                                out=oh[:, :n * 128].rearrange(
                                    "p (c w) -> p c w", w=128),
                                in0=dst_sb[:, :n][:, :, None].to_broadcast(
                                    [128, n, 128]),
                                in1=iota_sb[:][:, None, :].to_broadcast(
                                    [128, n, 128]),
                                op=EQ)
                            cur = dict(msgs=msgs, oh=oh)
                        elif op[0] == "mm":
                            _, si, kk, g, st, sp = op
                            sl = (g % BANKW) * PBANKF
                            nc.tensor.matmul(
                                out=psum_agg[:, sl:sl + h],
                                lhsT=cur["oh"][:, kk * 128:(kk + 1) * 128],
                                rhs=cur["msgs"][:, kk * h:(kk + 1) * h],
                                start=st, stop=sp)
                        elif op[0] == "flush0":
                            k = op[1]
                            g_lo = k * BANKW
                            g_hi = min(g_lo + BANKW, groups)
                            ng = g_hi - g_lo
                            nc.vector.tensor_copy(
                                agg[:, g_lo * h:g_hi * h].rearrange(
                                    "p (s w) -> p s w", w=h),
                                psum_agg[:, :ng * PBANKF].rearrange(
                                    "p (s w) -> p s w", w=PBANKF)[:, :, :h])
                        elif op[0] == "flushw":
                            k = op[1]
                            g_lo = k * BANKW
                            g_hi = min(g_lo + BANKW, groups)
                            ng = g_hi - g_lo
                            nc.vector.tensor_tensor(
                                out=agg[:, g_lo * h:g_hi * h].rearrange(
                                    "p (s w) -> p s w", w=h),
                                in0=agg[:, g_lo * h:g_hi * h].rearrange(
                                    "p (s w) -> p s w", w=h),
                                in1=psum_agg[:, :ng * PBANKF].rearrange(
                                    "p (s w) -> p s w", w=PBANKF)[:, :, :h],
                                op=ADD)
                            nc.vector.tensor_tensor(
                                out=agg[:, g_lo * h:g_hi * h]
                                .rearrange("p (g f) -> p g f", f=h),
                                in0=agg[:, g_lo * h:g_hi * h]
                                .rearrange("p (g f) -> p g f", f=h),
                                in1=winv_sb[:, g_lo:g_hi][:, :, None]
                                .to_broadcast([128, ng, h]),
                                op=MUL)
                        elif op[0] == "post":
                            k = op[1]
                            g_lo = k * BANKW
                            g_hi = min(g_lo + BANKW, groups)
                            ng = g_hi - g_lo
                            if debug_dump and L == 0:
                                nc.sync.dma_start(
                                    dbg_mean.ap()
                                    [g_lo * 128:g_hi * 128, :]
                                    .rearrange("(t p) f -> p t f", p=128),
                                    agg[:, g_lo * h:g_hi * h].rearrange(
                                        "p (t f) -> p t f", f=h))
                            xin = pio3.tile([128, BANKW * h], F32, tag="xin")
                            nc.sync.dma_start(
                                xin[:, :ng * h].rearrange(
                                    "p (t f) -> p t f", f=h),
                                x_own[L][g_lo * 128:g_hi * 128, :]
                                .rearrange("(t p) f -> p t f", p=128))
                            for t in range(g_lo, g_hi):
                                tl = t - g_lo
                                tp1 = psp.tile([64, 128], F32, tag="tp")
                                nc.tensor.transpose(
                                    out=tp1[:h, :],
                                    in_=agg[:, t * h:(t + 1) * h],
                                    identity=ident[:])
                                aggT = ppo.tile([h, 128], F32, tag="aggT")
                                nc.scalar.copy(aggT[:], tp1[:h, :])
                                tp2 = psp.tile([64, 128], F32, tag="tp")
                                nc.tensor.transpose(
                                    out=tp2[:h, :],
                                    in_=xin[:, tl * h:(tl + 1) * h],
                                    identity=ident[:])
                                xT = ppo.tile([h, 128], F32, tag="xT")
                                nc.scalar.copy(xT[:], tp2[:h, :])
                                ym = psp.tile([128, h], F32, tag="ym")
                                nc.tensor.matmul(out=ym[:], lhsT=aggT[:],
                                                 rhs=wl_sb[L][:],
                                                 start=True, stop=False)
                                nc.tensor.matmul(out=ym[:], lhsT=xT[:],
                                                 rhs=wr_sb[L][:],
                                                 start=False, stop=True)
                                tmp = ppo.tile([128, h], F32, tag="tmp")
                                nc.vector.tensor_tensor(
                                    out=tmp[:], in0=ym[:],
                                    in1=brep_sb[L][:], op=ADD)
                                nc.scalar.activation(
                                    out=agg[:, t * h:(t + 1) * h],
                                    in_=tmp[:], func=RELU)
                            if L == 0:
                                nc.sync.dma_start(
                                    x_own1[g_lo * 128:g_hi * 128, :]
                                    .rearrange("(t p) f -> p t f", p=128),
                                    agg[:, g_lo * h:g_hi * h].rearrange(
                                        "p (t f) -> p t f", f=h))
                            else:
                                # classifier for this bank
                                for t in range(g_lo, g_hi):
                                    tp1 = psp.tile([64, 128], F32,
                                                   tag="tp")
                                    nc.tensor.transpose(
                                        out=tp1[:h, :],
                                        in_=agg[:, t * h:(t + 1) * h],
                                        identity=ident[:])
                                    x2T = ppo.tile([h, 128], F32, tag="x2T")
                                    nc.scalar.copy(x2T[:], tp1[:h, :])
                                    hps = psp.tile([128, h], F32,
                                                   tag="ym")
                                    nc.tensor.matmul(out=hps[:, :32],
                                                     lhsT=x2T[:],
                                                     rhs=wc1_sb[:],
                                                     start=True, stop=True)
                                    htmp = ppo.tile([128, 32], F32,
                                                    tag="htmp")
                                    nc.vector.tensor_tensor(
                                        out=htmp[:], in0=hps[:, :32],
                                        in1=bc1_sb[:], op=ADD)
                                    hsb = ppo.tile([128, 32], F32, tag="hsb")
                                    nc.scalar.activation(out=hsb[:],
                                                         in_=htmp[:],
                                                         func=RELU)
                                    tp2 = psp.tile([64, 128], F32,
                                                   tag="tp")
                                    nc.tensor.transpose(out=tp2[:32, :],
                                                        in_=hsb[:],
                                                        identity=ident[:])
                                    hT = ppo.tile([32, 128], F32, tag="hT")
                                    nc.scalar.copy(hT[:], tp2[:32, :])
                                    ops_ = psp.tile([128, h], F32,
                                                    tag="ym")
                                    nc.tensor.matmul(out=ops_[:, :2],
                                                     lhsT=hT[:],
                                                     rhs=wc2_sb[:],
                                                     start=True, stop=True)
                                    nc.vector.tensor_tensor(
                                        out=outb[:, t * 2:(t + 1) * 2],
                                        in0=ops_[:, :2], in1=bc2_sb[:],
                                        op=ADD)

                if L == 0:
                    nc.gpsimd.collective_compute(
                        AG, BYP, replica_groups=[list(range(cores))],
                        ins=[x_own1[:, :]], outs=[xf1[:, :]])
                    if debug_dump:
                        for b in range(nblk):
                            nc.sync.dma_start(
                                dbg_xf0.ap()[b * blk:b * blk + blk_rows[b], :],
                                xf0[b][:, :])
                        nc.sync.dma_start(dbg_xo1.ap(), x_own1[:, :])
                        nc.sync.dma_start(dbg_xo0.ap(), x_own0[:, :])

            nc.sync.dma_start(
                out_d.ap().rearrange("(t p) f -> p t f", p=128),
                outb[:].rearrange("p (t f) -> p t f", f=2))

    nc.compile()
    return nc


def run(cfg, inputs, trace=False, debug_dump=False):
    d = derive(cfg)
    s = plan(d, inputs["edge_index"])
    in_maps = core_inputs(
        s, **{k: v for k, v in inputs.items() if k != "edge_index"})
    nc = build_program(s, debug_dump=debug_dump)
    res = run_bass_kernel_spmd(nc, in_maps, core_ids=list(range(d["cores"])),
                               trace=trace)
    outs = [res.results[c]["out"] for c in range(d["cores"])]
    out_full = np.concatenate(outs, axis=0)  # [nptot, 2]
    final = out_full[s.perm]                 # original node order
    return final.astype(np.float32), res


def kernel(**inputs):
    out, _ = run(FULL_CFG, inputs)
    return out


# revision 17
# speedup vs baseline: 1.2441x; 1.2441x over previous
"""Trainium2 Bass kernel for a 3-type heterogeneous GraphSAGE GNN.

Full-input contract: kernel(**inputs) takes the unsharded numpy inputs and
returns the full [300000, 2] output. Internally:
  - Nodes are relabeled so each of the 8 cores owns a contiguous range of
    37632 padded nodes (12544 per type); edges are sharded by dst owner.
  - Per core, edges are sorted by (src block of 32768, dst) and padded into
    a schedule of 128-edge chunks that is *uniform across cores* (the NEFF
    is SPMD — one program, per-core data).
  - Aggregation: dma_gather pulls x[src] rows (256B) from a replicated
    x_full in DRAM; a 0/1 one-hot [128 edges x 128 dsts] built on DVE via
    is_equal(iota, dstrel) turns segment-sum into PE matmuls accumulated
    in PSUM.
  - Schedule: pass 1 covers block 0 for all banks (runs while the
    replicated encoder still produces later blocks); pass 2 is bank-outer
    (for bank: for blocks 1..9) so each bank of 8 dst-groups finishes
    early and its mean/post-linear/classifier work pipelines on PE/DVE/Act
    underneath the GpSimd gather stream.
  - Layer-0 x_full is produced by a replicated encoder (every core encodes
    all nodes; no collective); layer-1 x_full needs one AllGather.
"""

import numpy as np

import concourse.bass as bass
import concourse.bacc as bacc
import concourse.mybir as mybir
import concourse.tile as tile
from concourse.masks import make_identity
from concourse.bass_utils import run_bass_kernel_spmd

F32 = mybir.dt.float32
I16 = mybir.dt.int16

FULL_CFG = dict(type_size=100000, E=4800000, cores=8, blk=32768,
                strip_chunks=16, h=64)

KENC = 49      # 48 padded features + ones column
BANKW = 4      # dst-groups per bank (one PSUM bank per in-flight chain)
PBANKF = 512   # f32 per PSUM bank per partition


def derive(cfg):
    cores = cfg["cores"]
    seg = cfg["type_size"] // cores          # real nodes per (core, type)
    assert seg * cores == cfg["type_size"]
    segp = -(-seg // 128) * 128              # padded to tile multiple
    npc = 3 * segp                           # nodes per core (padded)
    nptot = cores * npc
    tiles = npc // 128
    groups = tiles                           # 128-dst groups per core
    nblk = -(-nptot // cfg["blk"])
    nbank = -(-groups // BANKW)
    d = dict(cfg)
    d.update(seg=seg, segp=segp, npc=npc, nptot=nptot, tiles=tiles,
             groups=groups, nblk=nblk, nbank=nbank)
    return d


def node_perm(d):
    """perm_of_orig[j] = padded-global id of original node j."""
    ts, seg, segp, npc = d["type_size"], d["seg"], d["segp"], d["npc"]
    j = np.arange(ts)
    core = j // seg
    local = j % seg
    parts = [core * npc + t * segp + local for t in range(3)]
    return np.concatenate(parts)


class Sched:
    pass


def plan(d, edge_index):
    """Build the uniform schedule + per-core edge data arrays."""
    cores, npc, nptot, blk = d["cores"], d["npc"], d["nptot"], d["blk"]
    groups, nblk, sc = d["groups"], d["nblk"], d["strip_chunks"]
    nbank = d["nbank"]

    perm = node_perm(d)
    src_p = perm[np.asarray(edge_index[0], dtype=np.int64)]
    dst_p = perm[np.asarray(edge_index[1], dtype=np.int64)]

    deg = np.bincount(dst_p, minlength=nptot).astype(np.float64)
    winv_full = (1.0 / np.maximum(deg, 1.0)).astype(np.float32)

    # per-core sorted edge arrays + per-(block, group) counts
    core_of = dst_p // npc
    per_core = []
    counts = np.zeros((cores, nblk, groups), np.int64)
    for c in range(cores):
        m = core_of == c
        es = src_p[m]
        ed = dst_p[m] - c * npc
        b = es // blk
        order = np.lexsort((ed, b))
        es, ed, b = es[order], ed[order], b[order]
        g = ed // 128
        np.add.at(counts[c], (b, g), 1)
        per_core.append((es, ed))

    nch = np.maximum(1, -(-counts.max(axis=0) // 128))  # [nblk, groups]

    # ---- chunk list in program order ----
    # pass 1: (b=0, bank k, g in bank, j)    -- block 0 for every bank
    # pass 2: (bank k, b=1..nblk-1, g, j)    -- bank-outer for the rest
    # A group's psum accumulation: start at (b=0, j=0); b=0 partials are
    # flushed (copied) to agg per bank; pass-2 restarts psum per bank and
    # the final fused flush adds psum into agg then multiplies by winv.
    assert nblk >= 2
    chunks = []  # (b, g, start, stop)
    for k in range(nbank):
        g_lo, g_hi = k * BANKW, min((k + 1) * BANKW, groups)
        for g in range(g_lo, g_hi):
            n = int(nch[0, g])
            for j in range(n):
                chunks.append((0, g, j == 0, j == n - 1))
    for k in range(nbank):
        g_lo, g_hi = k * BANKW, min((k + 1) * BANKW, groups)
        for b in range(1, nblk):
            for g in range(g_lo, g_hi):
                n = int(nch[b, g])
                for j in range(n):
                    chunks.append((b, g, b == 1 and j == 0,
                                   b == nblk - 1 and j == n - 1))
    nchunks = len(chunks)
    chunk_b = np.array([c[0] for c in chunks])
    chunk_g = np.array([c[1] for c in chunks])

    # ---- strips: runs of consecutive chunks sharing b, cut at sc ----
    strips = []  # (b, c0, n, idx_col_off)
    idx_off = 0
    c0 = 0
    for ci in range(nchunks + 1):
        if ci == nchunks or (ci > c0 and chunk_b[ci] != chunk_b[c0]) \
           or ci - c0 == sc:
            if ci > c0:
                n = ci - c0
                strips.append((int(chunk_b[c0]), c0, n, idx_off))
                idx_off += n * 8
            c0 = ci
    idx_cols = idx_off
    strip_of_chunk = np.zeros(nchunks, np.int64)
    strip_c0 = np.zeros(nchunks, np.int64)
    for si, (b, sc0, n, io) in enumerate(strips):
        strip_of_chunk[sc0:sc0 + n] = si
        strip_c0[sc0:sc0 + n] = sc0

    # ---- op list: strips/mms/bank-end markers in program order ----
    # ops: ("strip", si) | ("mm", si, k_in_strip, g, start, stop)
    #      | ("flush0", bank)  -- after pass-1 bank: copy psum -> agg
    #      | ("flushw", bank)  -- after pass-2 bank: agg=(agg+psum)*winv
    #      | ("post", bank)    -- post-linear for the bank (layer-specific)
    ops = []
    for si, (b, sc0, n, io) in enumerate(strips):
        ops.append(("strip", si))
        for kk in range(n):
            cb, cg, cst, csp = chunks[sc0 + kk]
            ops.append(("mm", si, kk, cg, cst, csp))
            nxt = sc0 + kk + 1
            bank = cg // BANKW
            g_hi = min((bank + 1) * BANKW, groups) - 1
            if cb == 0 and csp:
                # pass-1: after last chunk of this bank's b=0 run
                is_end = (nxt == nchunks or chunk_b[nxt] != 0
                          or chunk_g[nxt] // BANKW != bank)
                if is_end:
                    ops.append(("flush0", bank))
            if cb == nblk - 1 and csp and cg == g_hi:
                ops.append(("flushw", bank))
                ops.append(("post", bank))

    # ---- per-core data arrays ----
    # slots: chunk ci occupies slots [ci*128, (ci+1)*128)
    # cell (b,g) slot ranges: consecutive chunks of the cell
    cell_first_chunk = {}
    for ci2, (b, g, st, sp) in enumerate(chunks):
        if st or (b, g) not in cell_first_chunk:
            pass
        if (b, g) not in cell_first_chunk:
            cell_first_chunk[(b, g)] = ci2
    # within-cell chunk index
    within_chunk = np.zeros(nchunks, np.int64)
    seen = {}
    for ci2, (b, g, st, sp) in enumerate(chunks):
        key = (b, g)
        within_chunk[ci2] = seen.get(key, 0)
        seen[key] = within_chunk[ci2] + 1

    slot = np.arange(nchunks * 128)
    ch_of_slot = slot // 128
    lane = slot % 128
    slot_b = chunk_b[ch_of_slot]
    slot_g = chunk_g[ch_of_slot]
    within = within_chunk[ch_of_slot] * 128 + lane   # position within cell

    strip_local = (ch_of_slot - strip_c0[ch_of_slot]) * 128 + lane
    idx_col = np.array([strips[s][3] for s in strip_of_chunk[ch_of_slot]]) \
        + strip_local // 16
    idx_row = strip_local % 16

    # per-cell edge start offsets in the per-core sorted arrays
    # (sorted order is b-major then dst => cell order (b, g) b-major)
    idx_all = np.zeros((cores, 128, idx_cols), np.int16)
    dstrel_all = np.full((cores, 128, nchunks), -1.0, np.float32)
    for c in range(cores):
        es, ed = per_core[c]
        ccounts = counts[c]                       # [nblk, groups]
        flat = ccounts.ravel()
        cell_start = np.concatenate([[0], np.cumsum(flat)])
        cell_id = slot_b * groups + slot_g
        cnt = flat[cell_id]
        real = within < cnt
        src_idx = cell_start[cell_id] + np.minimum(
            within, np.maximum(cnt - 1, 0))
        esv = np.where(real, es[np.minimum(src_idx, max(len(es) - 1, 0))]
                       if len(es) else 0, 0)
        edv = np.where(real, ed[np.minimum(src_idx, max(len(ed) - 1, 0))]
                       if len(ed) else 0, -1)
        rel = np.where(real, esv - slot_b * blk, 0).astype(np.int64)
        assert rel.min() >= 0 and rel.max() < blk
        drel = np.where(real, edv - slot_g * 128, -1.0).astype(np.float32)
        for r in range(8):
            idx_all[c, idx_row + 16 * r, idx_col] = rel.astype(np.int16)
        dstrel_all[c, lane, ch_of_slot] = drel

    s = Sched()
    s.d = d
    s.perm = perm
    s.strips = strips
    s.ops = ops
    s.nchunks = nchunks
    s.idx_cols = idx_cols
    s.winv_full = winv_full
    s.idx_all = idx_all
    s.dstrel_all = dstrel_all
    return s


def core_inputs(s, x_individual, x_company, x_trust,
                W_ind, b_ind, W_com, b_com, W_tru, b_tru,
                W1l, W1r, b1, W2l, W2r, b2, Wc1, bc1, Wc2, bc2):
    d = s.d
    cores, seg, segp, npc, groups = \
        d["cores"], d["seg"], d["segp"], d["npc"], d["groups"]
    raws = [np.asarray(x_individual, np.float32),
            np.asarray(x_company, np.float32),
            np.asarray(x_trust, np.float32)]
    Ws = [np.asarray(W_ind, np.float32), np.asarray(W_com, np.float32),
          np.asarray(W_tru, np.float32)]
    bs = [np.asarray(b_ind, np.float32), np.asarray(b_com, np.float32),
          np.asarray(b_tru, np.float32)]
    h = d["h"]

    w_enc = []
    for t in range(3):
        w = np.zeros((KENC, h), np.float32)
        w[:Ws[t].shape[0], :] = Ws[t]
        w[48, :] = bs[t]
        w_enc.append(w)

    # full padded raw features in permuted layout [nptot, KENC]
    nptot = d["nptot"]
    xraw_full = np.zeros((nptot, KENC), np.float32)
    for c in range(cores):
        for t in range(3):
            r0 = c * npc + t * segp
            xraw_full[r0:r0 + seg, :raws[t].shape[1]] = \
                raws[t][c * seg:(c + 1) * seg]
            xraw_full[r0:r0 + seg, 48] = 1.0

    shared = {
        "w_enc0": w_enc[0], "w_enc1": w_enc[1], "w_enc2": w_enc[2],
        "w1l": np.asarray(W1l, np.float32), "w1r": np.asarray(W1r, np.float32),
        "w2l": np.asarray(W2l, np.float32), "w2r": np.asarray(W2r, np.float32),
        "wc1": np.asarray(Wc1, np.float32), "wc2": np.asarray(Wc2, np.float32),
        "b1_rep": np.tile(np.asarray(b1, np.float32)[None, :], (128, 1)),
        "b2_rep": np.tile(np.asarray(b2, np.float32)[None, :], (128, 1)),
        "bc1_rep": np.tile(np.asarray(bc1, np.float32)[None, :], (128, 1)),
        "bc2_rep": np.tile(np.asarray(bc2, np.float32)[None, :], (128, 1)),
        "iota_rep": np.tile(np.arange(128, dtype=np.float32)[None, :],
                            (128, 1)),
    }

    in_maps = []
    for c in range(cores):
        xraw = np.zeros((npc, KENC), np.float32)
        for t in range(3):
            r0 = t * segp
            xraw[r0:r0 + seg, :raws[t].shape[1]] = \
                raws[t][c * seg:(c + 1) * seg]
            xraw[r0:r0 + seg, 48] = 1.0
        winv = s.winv_full[c * npc:(c + 1) * npc] \
            .reshape(groups, 128).T.copy()
        m = dict(shared)
        m.update(xraw=xraw, idx=s.idx_all[c], dstrel=s.dstrel_all[c],
                 winv=winv)
        in_maps.append(m)
    return in_maps


def build_program(s, debug_dump=False):
    d = s.d
    cores, npc, nptot, blk = d["cores"], d["npc"], d["nptot"], d["blk"]
    tiles, groups, nblk, h = d["tiles"], d["groups"], d["nblk"], d["h"]
    nbank, sc = d["nbank"], d["strip_chunks"]

    nc = bacc.Bacc("TRN2", target_bir_lowering=False, debug=False,
                   num_devices=cores, dynamic_dma_scratch_size=32768)

    di = {}
    def inp(name, shape, dt=F32):
        di[name] = nc.dram_tensor(name, list(shape), dt, kind="ExternalInput")
        return di[name]

    inp("xraw", [npc, KENC])
    inp("idx", [128, s.idx_cols], I16)
    inp("dstrel", [128, s.nchunks])
    inp("winv", [128, groups])
    inp("iota_rep", [128, 128])
    for t in range(3):
        inp(f"w_enc{t}", [KENC, h])
    inp("w1l", [h, h]); inp("w1r", [h, h]); inp("b1_rep", [128, h])
    inp("w2l", [h, h]); inp("w2r", [h, h]); inp("b2_rep", [128, h])
    inp("wc1", [h, 32]); inp("bc1_rep", [128, 32])
    inp("wc2", [32, 2]); inp("bc2_rep", [128, 2])
    out_d = nc.dram_tensor("out", [npc, 2], F32, kind="ExternalOutput")
    dbg_xf0 = dbg_xo1 = None
    if debug_dump:
        dbg_xf0 = nc.dram_tensor("dbg_xf0", [nptot, h], F32,
                                 kind="ExternalOutput")
        dbg_xo1 = nc.dram_tensor("dbg_xo1", [npc, h], F32,
                                 kind="ExternalOutput")
        dbg_xo0 = nc.dram_tensor("dbg_xo0", [npc, h], F32,
                                 kind="ExternalOutput")
        dbg_mean = nc.dram_tensor("dbg_mean", [npc, h], F32,
                                  kind="ExternalOutput")

    AG = "AllGather"
    ADD = mybir.AluOpType.add
    MUL = mybir.AluOpType.mult
    EQ = mybir.AluOpType.is_equal
    BYP = mybir.AluOpType.bypass
    RELU = mybir.ActivationFunctionType.Relu

    blk_rows = [min(blk, nptot - b * blk) for b in range(nblk)]

    with tile.TileContext(nc) as tc:
        with tc.tile_pool(name="persist", bufs=1) as pp, \
             tc.tile_pool(name="dram", bufs=1, space="DRAM") as dramp:
            # constants to SBUF
            def csb(name, shape, dt=F32):
                t_ = pp.tile(list(shape), dt, tag=name)
                nc.sync.dma_start(t_[:], di[name].ap())
                return t_
            iota_sb = csb("iota_rep", [128, 128])
            winv_sb = csb("winv", [128, groups])
            wenc_sb = [csb(f"w_enc{t}", [KENC, h]) for t in range(3)]
            wl_sb = [csb("w1l", [h, h]), csb("w2l", [h, h])]
            wr_sb = [csb("w1r", [h, h]), csb("w2r", [h, h])]
            brep_sb = [csb("b1_rep", [128, h]), csb("b2_rep", [128, h])]
            wc1_sb = csb("wc1", [h, 32])
            bc1_sb = csb("bc1_rep", [128, 32])
            wc2_sb = csb("wc2", [32, 2])
            bc2_sb = csb("bc2_rep", [128, 2])
            ident = pp.tile([128, 128], F32, tag="ident")
            make_identity(nc, ident[:])

            agg = pp.tile([128, groups * h], F32, tag="agg")
            outb = pp.tile([128, tiles * 2], F32, tag="outb")

            x_own0 = dramp.tile([npc, h], F32)
            x_own1 = dramp.tile([npc, h], F32)
            xf0 = dramp.tile([nptot, h], F32, name="xf0")
            xf1 = dramp.tile([nptot, h], F32, name="xf1")

            segp = d["segp"]
            # ---------------- own-shard encoder -> x_own0 ----------------
            with tc.tile_pool(name="enc2io", bufs=1) as pio2, \
                 tc.tile_pool(name="enc2", bufs=4) as pe2, \
                 tc.tile_pool(name="enc2ps", bufs=4, space="PSUM") as pse2:
                xraw_sb = pio2.tile([128, tiles * KENC], F32)
                nc.sync.dma_start(
                    xraw_sb[:].rearrange("p (t f) -> p t f", f=KENC),
                    di["xraw"].ap().rearrange("(t p) f -> p t f", p=128))
                seg_tiles = segp // 128
                for t in range(tiles):
                    wseg = wenc_sb[t // seg_tiles]
                    tp = pse2.tile([64, 128], F32, tag="tp")
                    nc.tensor.transpose(
                        out=tp[:KENC, :],
                        in_=xraw_sb[:, t * KENC:(t + 1) * KENC],
                        identity=ident[:])
                    xrT = pe2.tile([KENC, 128], F32, tag="xrT")
                    nc.scalar.copy(xrT[:], tp[:KENC, :])
                    ym = pse2.tile([128, h], F32, tag="ym")
                    nc.tensor.matmul(out=ym[:], lhsT=xrT[:], rhs=wseg[:],
                                     start=True, stop=True)
                    nc.scalar.copy(agg[:, t * h:(t + 1) * h], ym[:])
                nc.sync.dma_start(
                    x_own0[:, :].rearrange("(t p) f -> p t f", p=128),
                    agg[:].rearrange("p (t f) -> p t f", f=h))
            nc.gpsimd.collective_compute(
                AG, BYP, replica_groups=[list(range(cores))],
                ins=[x_own0[:, :]], outs=[xf0[:, :]])

            # ---------------- SAGE layers ----------------
            x_own = [x_own0, x_own1]
            for L in range(2):
                def in_ap_of(b):
                    xsrc = xf0 if L == 0 else xf1
                    return xsrc[b * blk:b * blk + blk_rows[b], :]

                with tc.tile_pool(name=f"sage{L}", bufs=4) as pa, \
                     tc.tile_pool(name=f"sageps{L}", bufs=1,
                                  space="PSUM") as psa, \
                     tc.tile_pool(name=f"post{L}", bufs=4) as ppo, \
                     tc.tile_pool(name=f"post{L}ps", bufs=1,
                                  space="PSUM") as psp, \
                     tc.tile_pool(name=f"post{L}io", bufs=2) as pio3:
                    psum_agg = psa.tile([128, BANKW * PBANKF], F32)
                    # pre-zero message buffers (stale SBUF could hold NaNs;
                    # one-hot zero-rows would still propagate 0*NaN)
                    msgs_bufs = [pa.tile([128, sc * h], F32, tag="msgs",
                                         name=f"msgsbuf{L}_{i}")
                                 for i in range(4)]
                    for mb in msgs_bufs:
                        nc.vector.memset(mb[:], 0.0)
                    cur = {}
                    mi = 0
                    for op in s.ops:
                        if op[0] == "strip":
                            si = op[1]
                            b, c0, n, ioff = s.strips[si]
                            idx_sb = pa.tile([128, sc * 8], I16, tag="idx")
                            nc.sync.dma_start(
                                idx_sb[:, :n * 8],
                                di["idx"].ap()[:, ioff:ioff + n * 8])
                            dst_sb = pa.tile([128, sc], F32, tag="dst")
                            nc.sync.dma_start(
                                dst_sb[:, :n],
                                di["dstrel"].ap()[:, c0:c0 + n])
                            msgs = msgs_bufs[mi % 4]
                            mi += 1
                            oh = pa.tile([128, sc * 128], F32, tag="oh")
                            nc.gpsimd.dma_gather(
                                out_ap=msgs[:, :n * h].rearrange(
                                    "p (c f) -> p c f", f=h),
                                in_ap=in_ap_of(b),
                                idxs_ap=idx_sb[:, :n * 8],
                                num_idxs=n * 128, num_idxs_reg=n * 128,
                                elem_size=h, single_packet=False)
                            nc.vector.tensor_tensor(
                                out=oh[:, :n * 128].rearrange(
                                    "p (c w) -> p c w", w=128),
                                in0=dst_sb[:, :n][:, :, None].to_broadcast(
                                    [128, n, 128]),
                                in1=iota_sb[:][:, None, :].to_broadcast(
                                    [128, n, 128]),
                                op=EQ)
                            cur = dict(msgs=msgs, oh=oh)
                        elif op[0] == "mm":
                            _, si, kk, g, st, sp = op
                            sl = (g % BANKW) * PBANKF
                            nc.tensor.matmul(
                                out=psum_agg[:, sl:sl + h],
                                lhsT=cur["oh"][:, kk * 128:(kk + 1) * 128],
                                rhs=cur["msgs"][:, kk * h:(kk + 1) * h],
                                start=st, stop=sp)
                        elif op[0] == "flush0":
                            k = op[1]
                            g_lo = k * BANKW
                            g_hi = min(g_lo + BANKW, groups)
                            ng = g_hi - g_lo
                            nc.vector.tensor_copy(
                                agg[:, g_lo * h:g_hi * h].rearrange(
                                    "p (s w) -> p s w", w=h),
                                psum_agg[:, :ng * PBANKF].rearrange(
                                    "p (s w) -> p s w", w=PBANKF)[:, :, :h])
                        elif op[0] == "flushw":
                            k = op[1]
                            g_lo = k * BANKW
                            g_hi = min(g_lo + BANKW, groups)
                            ng = g_hi - g_lo
                            nc.vector.tensor_tensor(
                                out=agg[:, g_lo * h:g_hi * h].rearrange(
                                    "p (s w) -> p s w", w=h),
                                in0=agg[:, g_lo * h:g_hi * h].rearrange(
                                    "p (s w) -> p s w", w=h),
                                in1=psum_agg[:, :ng * PBANKF].rearrange(
                                    "p (s w) -> p s w", w=PBANKF)[:, :, :h],
                                op=ADD)
                            nc.vector.tensor_tensor(
                                out=agg[:, g_lo * h:g_hi * h]
                                .rearrange("p (g f) -> p g f", f=h),
                                in0=agg[:, g_lo * h:g_hi * h]
                                .rearrange("p (g f) -> p g f", f=h),
                                in1=winv_sb[:, g_lo:g_hi][:, :, None]
                                .to_broadcast([128, ng, h]),
                                op=MUL)
                        elif op[0] == "post":
                            k = op[1]
                            g_lo = k * BANKW
                            g_hi = min(g_lo + BANKW, groups)
                            ng = g_hi - g_lo
                            if debug_dump and L == 0:
                                nc.sync.dma_start(
                                    dbg_mean.ap()
                                    [g_lo * 128:g_hi * 128, :]
                                    .rearrange("(t p) f -> p t f", p=128),
                                    agg[:, g_lo * h:g_hi * h].rearrange(
                                        "p (t f) -> p t f", f=h))
                            xin = pio3.tile([128, BANKW * h], F32, tag="xin")
                            nc.sync.dma_start(
                                xin[:, :ng * h].rearrange(
                                    "p (t f) -> p t f", f=h),
                                x_own[L][g_lo * 128:g_hi * 128, :]
                                .rearrange("(t p) f -> p t f", p=128))
                            for t in range(g_lo, g_hi):
                                tl = t - g_lo
                                tp1 = psp.tile([64, 128], F32, tag="tp")
                                nc.tensor.transpose(
                                    out=tp1[:h, :],
                                    in_=agg[:, t * h:(t + 1) * h],
                                    identity=ident[:])
                                aggT = ppo.tile([h, 128], F32, tag="aggT")
                                nc.scalar.copy(aggT[:], tp1[:h, :])
                                tp2 = psp.tile([64, 128], F32, tag="tp")
                                nc.tensor.transpose(
                                    out=tp2[:h, :],
                                    in_=xin[:, tl * h:(tl + 1) * h],
                                    identity=ident[:])
                                xT = ppo.tile([h, 128], F32, tag="xT")
                                nc.scalar.copy(xT[:], tp2[:h, :])
                                ym = psp.tile([128, h], F32, tag="ym")
                                nc.tensor.matmul(out=ym[:], lhsT=aggT[:],
                                                 rhs=wl_sb[L][:],
                                                 start=True, stop=False)
                                nc.tensor.matmul(out=ym[:], lhsT=xT[:],
                                                 rhs=wr_sb[L][:],
                                                 start=False, stop=True)
                                tmp = ppo.tile([128, h], F32, tag="tmp")
                                nc.vector.tensor_tensor(
                                    out=tmp[:], in0=ym[:],
                                    in1=brep_sb[L][:], op=ADD)
                                nc.scalar.activation(
                                    out=agg[:, t * h:(t + 1) * h],
                                    in_=tmp[:], func=RELU)
                            if L == 0:
                                nc.sync.dma_start(
                                    x_own1[g_lo * 128:g_hi * 128, :]
                                    .rearrange("(t p) f -> p t f", p=128),
                                    agg[:, g_lo * h:g_hi * h].rearrange(
                                        "p (t f) -> p t f", f=h))
                            else:
                                # classifier for this bank
                                for t in range(g_lo, g_hi):
                                    tp1 = psp.tile([64, 128], F32,
                                                   tag="tp")
                                    nc.tensor.transpose(
                                        out=tp1[:h, :],
                                        in_=agg[:, t * h:(t + 1) * h],
                                        identity=ident[:])
                                    x2T = ppo.tile([h, 128], F32, tag="x2T")
                                    nc.scalar.copy(x2T[:], tp1[:h, :])
                                    hps = psp.tile([128, h], F32,
                                                   tag="ym")
                                    nc.tensor.matmul(out=hps[:, :32],
                                                     lhsT=x2T[:],
                                                     rhs=wc1_sb[:],
                                                     start=True, stop=True)
                                    htmp = ppo.tile([128, 32], F32,
                                                    tag="htmp")
                                    nc.vector.tensor_tensor(
                                        out=htmp[:], in0=hps[:, :32],
                                        in1=bc1_sb[:], op=ADD)
                                    hsb = ppo.tile([128, 32], F32, tag="hsb")
                                    nc.scalar.activation(out=hsb[:],
                                                         in_=htmp[:],
                                                         func=RELU)
                                    tp2 = psp.tile([64, 128], F32,
                                                   tag="tp")
                                    nc.tensor.transpose(out=tp2[:32, :],
                                                        in_=hsb[:],
                                                        identity=ident[:])
                                    hT = ppo.tile([32, 128], F32, tag="hT")
                                    nc.scalar.copy(hT[:], tp2[:32, :])
                                    ops_ = psp.tile([128, h], F32,
                                                    tag="ym")
                                    nc.tensor.matmul(out=ops_[:, :2],
                                                     lhsT=hT[:],
                                                     rhs=wc2_sb[:],
                                                     start=True, stop=True)
                                    nc.vector.tensor_tensor(
                                        out=outb[:, t * 2:(t + 1) * 2],
                                        in0=ops_[:, :2], in1=bc2_sb[:],
                                        op=ADD)

                if L == 0:
                    nc.gpsimd.collective_compute(
                        AG, BYP, replica_groups=[list(range(cores))],
                        ins=[x_own1[:, :]], outs=[xf1[:, :]])
                    if debug_dump:
                        nc.sync.dma_start(dbg_xf0.ap(), xf0[:, :])
                        nc.sync.dma_start(dbg_xo1.ap(), x_own1[:, :])
                        nc.sync.dma_start(dbg_xo0.ap(), x_own0[:, :])

            nc.sync.dma_start(
                out_d.ap().rearrange("(t p) f -> p t f", p=128),
                outb[:].rearrange("p (t f) -> p t f", f=2))

    nc.compile()
    return nc


def run(cfg, inputs, trace=False, debug_dump=False):
    d = derive(cfg)
    s = plan(d, inputs["edge_index"])
    in_maps = core_inputs(
        s, **{k: v for k, v in inputs.items() if k != "edge_index"})
    nc = build_program(s, debug_dump=debug_dump)
    res = run_bass_kernel_spmd(nc, in_maps, core_ids=list(range(d["cores"])),
                               trace=trace)
    outs = [res.results[c]["out"] for c in range(d["cores"])]
    out_full = np.concatenate(outs, axis=0)  # [nptot, 2]
    final = out_full[s.perm]                 # original node order
    return final.astype(np.float32), res


def kernel(**inputs):
    out, _ = run(FULL_CFG, inputs)
    return out
